# revision 18
# baseline (speedup 1.0000x reference)
"""Additive (Bahdanau) cross-attention kernel for 8 TRN2 NeuronCores.

Math: scores[b,q,k] = sum_h v[h] * tanh(qh[b,q,h] + kh[b,k,h])
      weights = softmax_k(scores); out = weights @ values

Key trick: tanh(z) ~= sum_j b_j * sin(w_j z) (Fourier sine series, max err
2.7e-3 on |z|<=5), and sin(w(qh+kh)) = sin(w qh)cos(w kh) + cos(w qh)sin(w kh)
separates per-(q,k) work into rank-H matmuls: the O(LQ*LK*H) tanh evaluations
become 2J TensorEngine matmuls plus O((LQ+LK)*H) sin/cos ACT-engine evals.

Sharding: batch (4) x query-half (2) -> 8 cores, keys/values replicated per
batch pair; no collectives.

Hardware quirk honored throughout: PE transpose (S3_LW) instructions carry at
most ONE semaphore wait, so every transpose's inputs (source tile, identity,
PSUM slot) must depend on a single engine -- all transpose sources are
DVE-produced bf16 tiles and transpose PSUM slots are freed by DVE copies.
"""

import numpy as np
from contextlib import ExitStack

import concourse.bass as bass
import concourse.mybir as mybir
import concourse.tile as tile
from concourse.bass_utils import run_bass_kernel_spmd
from concourse.masks import make_identity

B, LQ, LK, D, H = 4, 256, 1024, 512, 128
QS = LQ // 2      # 128 queries per core
NCORE = 8
DCH = D // 128    # 4 contraction chunks
KT = LK // 128    # 8 key tiles

# tanh(z) ~= sum_j BCOEF[j] * sin(GRID[j]*OMEGA1*z); maxerr 2.7e-2, rms@data
# 1.6e-3.  Only GRID 1,2,3 are evaluated by the ACT Sin table (args stay
# within its accurate |x|<~pi window); 4, 6, 8 come from exact double-angle
# products.  GAMMA[j] compensates the 1/2-per-doubling in the product tiles.
OMEGA1 = 0.4487989505128276
GRID = [1, 2, 3, 4, 6, 8]
BCOEF = [1.1499596596, 0.0461030978, 0.1584585002, 0.0631930252,
         0.033470942, 0.0043143511]
GAMMA = {1: 1.0, 2: 1.0, 3: 1.0, 4: 2.0, 6: 2.0, 8: 4.0}
DERIVED = {4: 2, 6: 3, 8: 4}   # freq -> source freq (doubling)
J = len(GRID)
HALF_PI = 1.5707963267948966

f32 = mybir.dt.float32
bf16 = mybir.dt.bfloat16

_CACHE = {}


def _build():
    nc = bass.Bass("TRN2")
    # Register pi/2 as an init-time const AP (like bass's built-in 0.0/1.0) so
    # activation(bias=HALF_PI) carries no runtime dependency -- instructions
    # here can hold at most one cross-engine semaphore wait.
    _hp = nc.alloc_sbuf_tensor("const-f32-halfpi", [128, 1], f32)
    nc.gpsimd.memset(_hp.ap(), HALF_PI)
    nc.const_aps.aps[(f32, HALF_PI)] = _hp.ap()
    nc.all_engine_barrier()
    # qw = [query_shard | Wq | Wk | v] packed host-side: one DMA, one sem lane
    d_qw = nc.dram_tensor("qw", [128, 3 * D + 1], f32, kind="ExternalInput")
    d_keys = nc.dram_tensor("keys", [LK, D], f32, kind="ExternalInput")
    d_vals = nc.dram_tensor("values", [LK, D], f32, kind="ExternalInput")
    # outs = [weights (LK) | out (D)] packed: one store DMA
    d_outs = nc.dram_tensor("outs", [QS, LK + D], f32, kind="ExternalOutput")

    Sin = mybir.ActivationFunctionType.Sin
    Exp = mybir.ActivationFunctionType.Exp
    Copy = mybir.ActivationFunctionType.Copy
    mult = mybir.AluOpType.mult
    add = mybir.AluOpType.add

    with tile.TileContext(nc) as tc, ExitStack() as ctx:
        const = ctx.enter_context(tc.tile_pool(name="const", bufs=1))
        ldp = ctx.enter_context(tc.tile_pool(name="ldp", bufs=2))
        persist = ctx.enter_context(tc.tile_pool(name="persist", bufs=1))
        harm_k = ctx.enter_context(tc.tile_pool(name="harm_k", bufs=1))
        harm_q = ctx.enter_context(tc.tile_pool(name="harm_q", bufs=1))
        tailp = ctx.enter_context(tc.tile_pool(name="tailp", bufs=1))
        ps_tr = ctx.enter_context(tc.tile_pool(name="ps_tr", bufs=2, space="PSUM"))
        ps_qh = ctx.enter_context(tc.tile_pool(name="ps_qh", bufs=1, space="PSUM"))
        ps_kh = ctx.enter_context(tc.tile_pool(name="ps_kh", bufs=2, space="PSUM"))
        ps_sc = ctx.enter_context(tc.tile_pool(name="ps_sc", bufs=2, space="PSUM"))
        ps_out = ctx.enter_context(tc.tile_pool(name="ps_out", bufs=1, space="PSUM"))

        sub = mybir.AluOpType.subtract

        # identity for PE transposes: DVE-stamped so transposes only wait on DVE
        id_gp = const.tile([128, 128], bf16, tag="id_gp")
        make_identity(nc, id_gp[:])
        id_bf = const.tile([128, 128], bf16, tag="id_bf")
        nc.vector.tensor_copy(id_bf[:], id_gp[:])

        # ---- input DMAs up front: qw + keys halves on SP ring, values on ACT
        qw_sb = const.tile([128, 3 * D + 1], f32, tag="qw_sb")
        nc.sync.dma_start(out=qw_sb[:], in_=d_qw[:])
        kf = [ldp.tile([128, 4, D], f32, tag=f"keysf{h}", name=f"keysf{h}")
              for h in range(2)]
        for h in range(2):
            nc.sync.dma_start(
                out=kf[h][:],
                in_=d_keys[h * 512:(h + 1) * 512, :].rearrange(
                    "(t p) d -> p t d", p=128))
        vf = ldp.tile([128, KT, D], f32, tag="valsf", name="valsf")
        nc.scalar.dma_start(out=vf[:],
                            in_=d_vals[:].rearrange("(t p) d -> p t d", p=128))

        # ACT-owned v so DVE folds merge their deps onto the ACT semaphore
        v_sb = const.tile([128, 1], f32, tag="v_sb")
        nc.scalar.copy(v_sb[:], qw_sb[:, 3 * D:3 * D + 1])

        def transpose_group(dst_copies, srcs):
            """PE-transpose up to 4 [128,128] bf16 blocks through one
            [128,512] bf16 PSUM tile; dst_copies = [(dst_ap, psum_slice_idx)]
            done with ONE DVE copy via caller-provided AP."""
            p = ps_tr.tile([128, 512], bf16, tag="tr", name="tr_p")
            for i, src_ap in enumerate(srcs):
                nc.tensor.transpose(p[:, i * 128:(i + 1) * 128], src_ap, id_bf[:])
            dst_copies(p)

        # ---- query / Wq / Wk: cast, transpose (contiguous dest: one copy) ----
        def load_transposed(idx, tag):
            src_bf = ldp.tile([128, D], bf16, tag=f"ldbf_{tag}", name=f"ldbf_{tag}")
            nc.vector.tensor_copy(src_bf[:], qw_sb[:, idx * D:(idx + 1) * D])
            dstT = persist.tile([128, DCH * 128], bf16, tag=tag, name=tag)
            transpose_group(
                lambda p: nc.vector.tensor_copy(dstT[:], p[:]),
                [src_bf[:, c * 128:(c + 1) * 128] for c in range(DCH)])
            return dstT

        queryT = load_transposed(0, "queryT")
        WqT = load_transposed(1, "WqT")
        WkT = load_transposed(2, "WkT")

        # ---- qhT projection + q-side harmonics (all early, small) ----
        qhT = ps_qh.tile([128, 128], f32, tag="qhT")
        for c in range(DCH):
            nc.tensor.matmul(qhT[:], WqT[:, c * 128:(c + 1) * 128],
                             queryT[:, c * 128:(c + 1) * 128],
                             start=(c == 0), stop=(c == DCH - 1))
        qt_s, qt_c = {}, {}
        for jf in (1, 2, 3):
            w = jf * OMEGA1
            s = harm_q.tile([128, 128], bf16, tag=f"sinq{jf}", name=f"sinq{jf}")
            nc.scalar.activation(s[:], qhT[:], Sin, bias=0.0, scale=w)
            c = harm_q.tile([128, 128], bf16, tag=f"cosq{jf}", name=f"cosq{jf}")
            nc.scalar.activation(c[:], qhT[:], Sin, bias=HALF_PI, scale=w)
            qt_s[jf], qt_c[jf] = s, c
        for jf, sf in DERIVED.items():
            g2 = -2.0 * GAMMA[sf] * GAMMA[sf]
            s = harm_q.tile([128, 128], bf16, tag=f"sdq{jf}", name=f"sdq{jf}")
            nc.vector.tensor_tensor(s[:], qt_s[sf][:], qt_c[sf][:], mult)
            c = harm_q.tile([128, 128], bf16, tag=f"cdq{jf}", name=f"cdq{jf}")
            nc.vector.tensor_tensor(c[:], qt_s[sf][:], qt_s[sf][:], mult)
            nc.vector.tensor_scalar(c[:], c[:], float(g2), 1.0, mult, add)
            qt_s[jf], qt_c[jf] = s, c
        lhs_s, lhs_c = {}, {}
        for j, jf in enumerate(GRID):
            bg = float(BCOEF[j] * GAMMA[jf])
            ls = harm_q.tile([128, 128], bf16, tag=f"lhs_s{jf}", name=f"lhs_s{jf}")
            nc.vector.tensor_scalar(ls[:], qt_s[jf][:], v_sb[:], bg, mult, mult)
            lc = harm_q.tile([128, 128], bf16, tag=f"lhs_c{jf}", name=f"lhs_c{jf}")
            nc.vector.tensor_scalar(lc[:], qt_c[jf][:], v_sb[:], bg, mult, mult)
            lhs_s[jf], lhs_c[jf] = ls, lc

        # ---- values: one cast into a single [128, KT, 512] bf16 tile ----
        vals_bf = persist.tile([128, KT, D], bf16, tag="vals_bf")
        nc.vector.tensor_copy(vals_bf[:], vf[:])

        # ---- per k-half pipeline: cast -> transpose -> khT -> trig -> MMs ----
        # keysT layout: [128, DCH, LK]; chunk c, key tile kt at [:, c, kt*128:]
        keysT = persist.tile([128, DCH, LK], bf16, tag="keysT")
        scores = [ps_sc.tile([128, 512], f32, tag="scores", name=f"scores{i}")
                  for i in range(2)]
        exp_f = tailp.tile([128, LK], f32, tag="exp_f")
        sums = [tailp.tile([128, 1], f32, tag=f"sum{kh}", name=f"sum{kh}")
                for kh in range(2)]
        for h in range(2):
            kf_bf = ldp.tile([128, 4, D], bf16, tag=f"keysf_bf{h}",
                             name=f"keysf_bf{h}")
            nc.vector.tensor_copy(kf_bf[:], kf[h][:])
            for tt in range(4):
                kt = h * 4 + tt
                transpose_group(
                    lambda p, kt=kt: nc.vector.tensor_copy(
                        keysT[:, :, kt * 128:(kt + 1) * 128], p[:].rearrange(
                            "p (c x) -> p c x", c=DCH)),
                    [kf_bf[:, tt, c * 128:(c + 1) * 128] for c in range(DCH)])
            sl = slice(h * 512, (h + 1) * 512)
            khT = ps_kh.tile([128, 512], f32, tag="khT", name=f"khT{h}")
            for c in range(DCH):
                nc.tensor.matmul(khT[:], WkT[:, c * 128:(c + 1) * 128],
                                 keysT[:, c, sl], start=(c == 0),
                                 stop=(c == DCH - 1))
            kt_s, kt_c = {}, {}
            for jf in (1, 2, 3):
                w = jf * OMEGA1
                s = harm_k.tile([128, 512], bf16, tag=f"sink{jf}_{h}",
                                name=f"sink{jf}_{h}")
                nc.scalar.activation(s[:], khT[:], Sin, bias=0.0, scale=w)
                c = harm_k.tile([128, 512], bf16, tag=f"cosk{jf}_{h}",
                                name=f"cosk{jf}_{h}")
                nc.scalar.activation(c[:], khT[:], Sin, bias=HALF_PI, scale=w)
                kt_s[jf], kt_c[jf] = s, c
            for jf, sf in DERIVED.items():
                g2 = -2.0 * GAMMA[sf] * GAMMA[sf]
                s = harm_k.tile([128, 512], bf16, tag=f"sdk{jf}_{h}",
                                name=f"sdk{jf}_{h}")
                nc.vector.tensor_tensor(s[:], kt_s[sf][:], kt_c[sf][:], mult)
                c = harm_k.tile([128, 512], bf16, tag=f"cdk{jf}_{h}",
                                name=f"cdk{jf}_{h}")
                nc.vector.tensor_tensor(c[:], kt_s[sf][:], kt_s[sf][:], mult)
                nc.vector.tensor_scalar(c[:], c[:], float(g2), 1.0, mult, add)
                kt_s[jf], kt_c[jf] = s, c
            for j, jf in enumerate(GRID):
                nc.tensor.matmul(scores[h][:], lhs_s[jf][:], kt_c[jf][:],
                                 start=(j == 0), stop=False)
                nc.tensor.matmul(scores[h][:], lhs_c[jf][:], kt_s[jf][:],
                                 start=False, stop=(j == J - 1))
            # softmax numerator per half (scores are O(1): no max needed)
            nc.scalar.activation(exp_f[:, sl], scores[h][:], Exp, bias=0.0,
                                 scale=1.0, accum_out=sums[h][:])

        # ---- tail: normalize, weights out, final matmul ----
        sumtot = tailp.tile([128, 1], f32, tag="sumtot")
        nc.vector.tensor_tensor(sumtot[:], sums[0][:], sums[1][:], add)
        recip = tailp.tile([128, 1], f32, tag="recip")
        nc.vector.reciprocal(recip[:], sumtot[:])

        # bf16 exp first: syncs DVE to ACT so later normalizes self-drain only
        exp_bf = tailp.tile([128, LK], bf16, tag="exp_bf")
        nc.vector.tensor_copy(exp_bf[:], exp_f[:])

        outs_sb = tailp.tile([128, LK + D], f32, tag="outs_sb")
        nc.vector.tensor_scalar(outs_sb[:, :LK], exp_f[:], recip[:], None, mult)

        outp = ps_out.tile([128, D], f32, tag="outp")
        wTs = []
        for g in range(2):
            wT = tailp.tile([128, 512], bf16, tag=f"wT{g}", name=f"wT{g}")
            transpose_group(
                lambda p, wT=wT: nc.vector.tensor_copy(wT[:], p[:]),
                [exp_bf[:, g * 512 + i * 128:g * 512 + (i + 1) * 128]
                 for i in range(4)])
            wTs.append(wT)
        for t in range(KT):
            nc.tensor.matmul(outp[:], wTs[t // 4][:, (t % 4) * 128:(t % 4 + 1) * 128],
                             vals_bf[:, t, :], start=(t == 0), stop=(t == KT - 1))
        nc.vector.tensor_scalar(outs_sb[:, LK:], outp[:], recip[:], None, mult)
        nc.sync.dma_start(out=d_outs[:], in_=outs_sb[:])

    return nc



def _wait_limit(inst):
    op = inst.get("opcode")
    if op == "Matmult":
        return 1 if inst.get("is_transpose") else 2
    return 1


def _split_excess_waits(raw):
    """Walrus enforces tiny per-instruction sync-wait budgets (1 for most ops,
    2 for Drain/regular Matmult). Tile sometimes emits more (notably the
    kernel-tail drain, which waits on every engine + DMA lane). Hoist the
    excess into preceding same-engine Drain instructions."""
    import json as _json
    d = _json.loads(raw)
    n_split = 0
    for fn in d.get("functions", []):
        for bb in fn.get("blocks", []):
            insts = bb.get("instructions", [])
            out = []
            for inst in insts:
                si = inst.get("sync_info") or {}
                waits = si.get("on_wait") or []
                lim = _wait_limit(inst)
                if len(waits) > lim:
                    excess, keep = waits[:-lim], waits[-lim:]
                    for i, wcmd in enumerate(excess):
                        n_split += 1
                        out.append({
                            "debug": inst.get("debug"),
                            "engine": inst["engine"],
                            "ins": [], "outs": [],
                            "name": f"{inst['name']}-ws{i}",
                            "opcode": "Drain",
                            "sync_info": {"on_wait": [wcmd]},
                        })
                    si["on_wait"] = keep
                    inst["sync_info"] = si
                out.append(inst)
            bb["instructions"] = out
    return _json.dumps(d).encode()


def _patch_json(nc):
    orig = nc.to_json_bytes

    def patched():
        return _split_excess_waits(orig())

    nc.to_json_bytes = patched


def _get_nc():
    if "nc" not in _CACHE:
        nc = _build()
        _patch_json(nc)
        _CACHE["nc"] = nc
    return _CACHE["nc"]


def _run(inputs, trace=False):
    nc = _get_nc()
    query = np.asarray(inputs["query"], dtype=np.float32)
    keys = np.asarray(inputs["keys"], dtype=np.float32)
    values = np.asarray(inputs["values"], dtype=np.float32)
    Wq = np.ascontiguousarray(np.asarray(inputs["Wq"], dtype=np.float32))
    Wk = np.ascontiguousarray(np.asarray(inputs["Wk"], dtype=np.float32))
    v = np.asarray(inputs["v"], dtype=np.float32)

    in_maps = []
    for c in range(NCORE):
        b, qh = c // 2, c % 2
        qw = np.concatenate(
            [query[b, qh * QS:(qh + 1) * QS, :], Wq, Wk, v.reshape(H, 1)], axis=1)
        in_maps.append({
            "qw": np.ascontiguousarray(qw),
            "keys": np.ascontiguousarray(keys[b]),
            "values": np.ascontiguousarray(values[b]),
        })
    res = run_bass_kernel_spmd(nc, in_maps, core_ids=list(range(NCORE)),
                               trace=trace)
    out = np.zeros((B, LQ, D), dtype=np.float32)
    wout = np.zeros((B, LQ, LK), dtype=np.float32)
    for c in range(NCORE):
        b, qh = c // 2, c % 2
        outs = res.results[c]["outs"]
        wout[b, qh * QS:(qh + 1) * QS, :] = outs[:, :LK]
        out[b, qh * QS:(qh + 1) * QS, :] = outs[:, LK:]
    return (out, wout), res


def kernel(query, keys, values, Wq, Wk, v):
    (out, wout), _ = _run(dict(query=query, keys=keys, values=values,
                               Wq=Wq, Wk=Wk, v=v))
    return (out, wout)


# revision 19
# speedup vs baseline: 1.1311x; 1.1311x over previous
"""Additive (Bahdanau) cross-attention kernel for 8 TRN2 NeuronCores.

Math: scores[b,q,k] = sum_h v[h] * tanh(qh[b,q,h] + kh[b,k,h])
      weights = softmax_k(scores); out = weights @ values

Key trick: tanh(z) ~= sum_j b_j * sin(w_j z) (Fourier sine series, max err
2.7e-3 on |z|<=5), and sin(w(qh+kh)) = sin(w qh)cos(w kh) + cos(w qh)sin(w kh)
separates per-(q,k) work into rank-H matmuls: the O(LQ*LK*H) tanh evaluations
become 2J TensorEngine matmuls plus O((LQ+LK)*H) sin/cos ACT-engine evals.

Sharding: batch (4) x query-half (2) -> 8 cores, keys/values replicated per
batch pair; no collectives.

Hardware quirk honored throughout: PE transpose (S3_LW) instructions carry at
most ONE semaphore wait, so every transpose's inputs (source tile, identity,
PSUM slot) must depend on a single engine -- all transpose sources are
DVE-produced bf16 tiles and transpose PSUM slots are freed by DVE copies.
"""

import numpy as np
from contextlib import ExitStack

import concourse.bass as bass
import concourse.mybir as mybir
import concourse.tile as tile
from concourse.bass_utils import run_bass_kernel_spmd
from concourse.masks import make_identity

B, LQ, LK, D, H = 4, 256, 1024, 512, 128
QS = LQ // 2      # 128 queries per core
NCORE = 8
DCH = D // 128    # 4 contraction chunks
KT = LK // 128    # 8 key tiles

# tanh(z) ~= sum_j BCOEF[j] * sin(GRID[j]*OMEGA1*z); maxerr 2.7e-2, rms@data
# 1.6e-3.  Only GRID 1,2,3 are evaluated by the ACT Sin table (args stay
# within its accurate |x|<~pi window); 4, 6, 8 come from exact double-angle
# products.  GAMMA[j] compensates the 1/2-per-doubling in the product tiles.
OMEGA1 = 0.4487989505128276
GRID = [1, 2, 3, 4, 6, 8]
BCOEF = [1.1499596596, 0.0461030978, 0.1584585002, 0.0631930252,
         0.033470942, 0.0043143511]
GAMMA = {1: 1.0, 2: 1.0, 3: 1.0, 4: 2.0, 6: 2.0, 8: 4.0}
DERIVED = {4: 2, 6: 3, 8: 4}   # freq -> source freq (doubling)
J = len(GRID)
HALF_PI = 1.5707963267948966

f32 = mybir.dt.float32
bf16 = mybir.dt.bfloat16

_CACHE = {}


def _build():
    nc = bass.Bass("TRN2")
    # qw = [query_shard | Wq | Wk | v] packed host-side: one DMA, one sem lane
    d_qw = nc.dram_tensor("qw", [128, 3 * D + 130], f32, kind="ExternalInput")
    d_keys = nc.dram_tensor("keys", [LK, D], f32, kind="ExternalInput")
    d_vals = nc.dram_tensor("values", [LK, D], f32, kind="ExternalInput")
    d_wout = nc.dram_tensor("wout", [QS, LK], f32, kind="ExternalOutput")
    d_out = nc.dram_tensor("out", [QS, D], f32, kind="ExternalOutput")

    Sin = mybir.ActivationFunctionType.Sin
    Exp = mybir.ActivationFunctionType.Exp
    Copy = mybir.ActivationFunctionType.Copy
    mult = mybir.AluOpType.mult
    add = mybir.AluOpType.add

    with tile.TileContext(nc) as tc, ExitStack() as ctx:
        const = ctx.enter_context(tc.tile_pool(name="const", bufs=1))
        ldp = ctx.enter_context(tc.tile_pool(name="ldp", bufs=2))
        persist = ctx.enter_context(tc.tile_pool(name="persist", bufs=1))
        harm_k = ctx.enter_context(tc.tile_pool(name="harm_k", bufs=1))
        harm_q = ctx.enter_context(tc.tile_pool(name="harm_q", bufs=1))
        tailp = ctx.enter_context(tc.tile_pool(name="tailp", bufs=1))
        ps_tr = ctx.enter_context(tc.tile_pool(name="ps_tr", bufs=2, space="PSUM"))
        ps_qh = ctx.enter_context(tc.tile_pool(name="ps_qh", bufs=1, space="PSUM"))
        ps_kh = ctx.enter_context(tc.tile_pool(name="ps_kh", bufs=2, space="PSUM"))
        ps_sc = ctx.enter_context(tc.tile_pool(name="ps_sc", bufs=2, space="PSUM"))
        ps_out = ctx.enter_context(tc.tile_pool(name="ps_out", bufs=1, space="PSUM"))

        sub = mybir.AluOpType.subtract

        # ---- input DMAs up front: keys halves on SP ring; qw + values on the
        # ACT ring (qw first: it gates the whole q side)
        kf = [ldp.tile([128, 4, D], f32, tag=f"keysf{h}", name=f"keysf{h}")
              for h in range(2)]
        for h in range(2):
            nc.sync.dma_start(
                out=kf[h][:],
                in_=d_keys[h * 512:(h + 1) * 512, :].rearrange(
                    "(t p) d -> p t d", p=128))
        qw_sb = const.tile([128, 3 * D + 130], f32, tag="qw_sb")
        nc.scalar.dma_start(out=qw_sb[:], in_=d_qw[:])
        vf = ldp.tile([128, KT, D], f32, tag="valsf", name="valsf")
        nc.scalar.dma_start(out=vf[:],
                            in_=d_vals[:].rearrange("(t p) d -> p t d", p=128))

        # identity for PE transposes arrives packed in qw (host constant);
        # DVE cast makes it DVE-owned for the single-wait transpose discipline
        id_bf = const.tile([128, 128], bf16, tag="id_bf")
        nc.vector.tensor_copy(id_bf[:], qw_sb[:, 3 * D + 1:3 * D + 129])
        halfpi_ap = qw_sb[:, 3 * D + 129:3 * D + 130]

        # ACT-owned v so DVE folds merge their deps onto the ACT semaphore
        v_sb = const.tile([128, 1], f32, tag="v_sb")
        nc.scalar.copy(v_sb[:], qw_sb[:, 3 * D:3 * D + 1])

        def transpose_group(dst_copies, srcs):
            """PE-transpose up to 4 [128,128] bf16 blocks through one
            [128,512] bf16 PSUM tile; dst_copies = [(dst_ap, psum_slice_idx)]
            done with ONE DVE copy via caller-provided AP."""
            p = ps_tr.tile([128, 512], bf16, tag="tr", name="tr_p")
            for i, src_ap in enumerate(srcs):
                nc.tensor.transpose(p[:, i * 128:(i + 1) * 128], src_ap, id_bf[:])
            dst_copies(p)

        # ---- query / Wq / Wk: cast, transpose (contiguous dest: one copy) ----
        def load_transposed(idx, tag):
            src_bf = ldp.tile([128, D], bf16, tag=f"ldbf_{tag}", name=f"ldbf_{tag}")
            nc.vector.tensor_copy(src_bf[:], qw_sb[:, idx * D:(idx + 1) * D])
            dstT = persist.tile([128, DCH * 128], bf16, tag=tag, name=tag)
            transpose_group(
                lambda p: nc.vector.tensor_copy(dstT[:], p[:]),
                [src_bf[:, c * 128:(c + 1) * 128] for c in range(DCH)])
            return dstT

        queryT = load_transposed(0, "queryT")
        WqT = load_transposed(1, "WqT")
        WkT = load_transposed(2, "WkT")

        # ---- qhT projection + q-side harmonics (all early, small) ----
        qhT = ps_qh.tile([128, 128], f32, tag="qhT")
        for c in range(DCH):
            nc.tensor.matmul(qhT[:], WqT[:, c * 128:(c + 1) * 128],
                             queryT[:, c * 128:(c + 1) * 128],
                             start=(c == 0), stop=(c == DCH - 1))
        qt_s, qt_c = {}, {}
        for jf in (1, 2, 3):
            w = jf * OMEGA1
            s = harm_q.tile([128, 128], bf16, tag=f"sinq{jf}", name=f"sinq{jf}")
            nc.scalar.activation(s[:], qhT[:], Sin, bias=0.0, scale=w)
            c = harm_q.tile([128, 128], bf16, tag=f"cosq{jf}", name=f"cosq{jf}")
            nc.scalar.activation(c[:], qhT[:], Sin, bias=halfpi_ap, scale=w)
            qt_s[jf], qt_c[jf] = s, c
        for jf, sf in DERIVED.items():
            g2 = -2.0 * GAMMA[sf] * GAMMA[sf]
            s = harm_q.tile([128, 128], bf16, tag=f"sdq{jf}", name=f"sdq{jf}")
            nc.vector.tensor_tensor(s[:], qt_s[sf][:], qt_c[sf][:], mult)
            c = harm_q.tile([128, 128], bf16, tag=f"cdq{jf}", name=f"cdq{jf}")
            nc.vector.tensor_tensor(c[:], qt_s[sf][:], qt_s[sf][:], mult)
            nc.vector.tensor_scalar(c[:], c[:], float(g2), 1.0, mult, add)
            qt_s[jf], qt_c[jf] = s, c
        lhs_s, lhs_c = {}, {}
        for j, jf in enumerate(GRID):
            bg = float(BCOEF[j] * GAMMA[jf])
            ls = harm_q.tile([128, 128], bf16, tag=f"lhs_s{jf}", name=f"lhs_s{jf}")
            nc.vector.tensor_scalar(ls[:], qt_s[jf][:], v_sb[:], bg, mult, mult)
            lc = harm_q.tile([128, 128], bf16, tag=f"lhs_c{jf}", name=f"lhs_c{jf}")
            nc.vector.tensor_scalar(lc[:], qt_c[jf][:], v_sb[:], bg, mult, mult)
            lhs_s[jf], lhs_c[jf] = ls, lc

        # ---- values: one cast into a single [128, KT, 512] bf16 tile ----
        vals_bf = persist.tile([128, KT, D], bf16, tag="vals_bf")
        nc.vector.tensor_copy(vals_bf[:], vf[:])

        # ---- per k-half pipeline: cast -> transpose -> khT -> trig -> MMs ----
        # keysT layout per half: [128, DCH, 512] (separate tiles so the two
        # half-pipelines carry no false tile-level dependencies)
        keysT = [persist.tile([128, DCH, 512], bf16, tag=f"keysT{h}",
                              name=f"keysT{h}") for h in range(2)]
        scores = [ps_sc.tile([128, 512], f32, tag="scores", name=f"scores{i}")
                  for i in range(2)]
        exp_f = tailp.tile([128, LK], f32, tag="exp_f")
        exp_bf = tailp.tile([128, LK], bf16, tag="exp_bf")
        sums = [tailp.tile([128, 1], f32, tag=f"sum{kh}", name=f"sum{kh}")
                for kh in range(2)]
        for h in range(2):
            kf_bf = ldp.tile([128, 4, D], bf16, tag=f"keysf_bf{h}",
                             name=f"keysf_bf{h}")
            nc.vector.tensor_copy(kf_bf[:], kf[h][:])
            for tt in range(4):
                transpose_group(
                    lambda p, tt=tt, h=h: nc.vector.tensor_copy(
                        keysT[h][:, :, tt * 128:(tt + 1) * 128], p[:].rearrange(
                            "p (c x) -> p c x", c=DCH)),
                    [kf_bf[:, tt, c * 128:(c + 1) * 128] for c in range(DCH)])
            sl = slice(h * 512, (h + 1) * 512)
            khT = ps_kh.tile([128, 512], f32, tag="khT", name=f"khT{h}")
            for c in range(DCH):
                nc.tensor.matmul(khT[:], WkT[:, c * 128:(c + 1) * 128],
                                 keysT[h][:, c, :], start=(c == 0),
                                 stop=(c == DCH - 1))
            kt_s, kt_c = {}, {}
            for jf in (1, 2, 3):
                w = jf * OMEGA1
                s = harm_k.tile([128, 512], bf16, tag=f"sink{jf}_{h}",
                                name=f"sink{jf}_{h}")
                nc.scalar.activation(s[:], khT[:], Sin, bias=0.0, scale=w)
                c = harm_k.tile([128, 512], bf16, tag=f"cosk{jf}_{h}",
                                name=f"cosk{jf}_{h}")
                nc.scalar.activation(c[:], khT[:], Sin, bias=halfpi_ap, scale=w)
                kt_s[jf], kt_c[jf] = s, c
            for jf, sf in DERIVED.items():
                g2 = -2.0 * GAMMA[sf] * GAMMA[sf]
                s = harm_k.tile([128, 512], bf16, tag=f"sdk{jf}_{h}",
                                name=f"sdk{jf}_{h}")
                nc.vector.tensor_tensor(s[:], kt_s[sf][:], kt_c[sf][:], mult)
                c = harm_k.tile([128, 512], bf16, tag=f"cdk{jf}_{h}",
                                name=f"cdk{jf}_{h}")
                nc.vector.tensor_tensor(c[:], kt_s[sf][:], kt_s[sf][:], mult)
                nc.vector.tensor_scalar(c[:], c[:], float(g2), 1.0, mult, add)
                kt_s[jf], kt_c[jf] = s, c
            for j, jf in enumerate(GRID):
                nc.tensor.matmul(scores[h][:], lhs_s[jf][:], kt_c[jf][:],
                                 start=(j == 0), stop=False)
                nc.tensor.matmul(scores[h][:], lhs_c[jf][:], kt_s[jf][:],
                                 start=False, stop=(j == J - 1))
            # softmax numerator per half (scores are O(1): no max needed)
            nc.scalar.activation(exp_f[:, sl], scores[h][:], Exp, bias=0.0,
                                 scale=1.0, accum_out=sums[h][:])
            nc.vector.tensor_copy(exp_bf[:, sl], exp_f[:, sl])

        # ---- tail: normalize, weights out, final matmul ----
        sumtot = tailp.tile([128, 1], f32, tag="sumtot")
        nc.vector.tensor_tensor(sumtot[:], sums[0][:], sums[1][:], add)
        recip = tailp.tile([128, 1], f32, tag="recip")
        nc.vector.reciprocal(recip[:], sumtot[:])

        wf_sb = tailp.tile([128, LK], f32, tag="wf_sb")
        nc.vector.tensor_scalar(wf_sb[:], exp_f[:], recip[:], None, mult)
        nc.scalar.dma_start(out=d_wout[:], in_=wf_sb[:])

        outp = ps_out.tile([128, D], f32, tag="outp")
        wTs = []
        for g in range(2):
            wT = tailp.tile([128, 512], bf16, tag=f"wT{g}", name=f"wT{g}")
            transpose_group(
                lambda p, wT=wT: nc.vector.tensor_copy(wT[:], p[:]),
                [exp_bf[:, g * 512 + i * 128:g * 512 + (i + 1) * 128]
                 for i in range(4)])
            wTs.append(wT)
        for t in range(KT):
            nc.tensor.matmul(outp[:], wTs[t // 4][:, (t % 4) * 128:(t % 4 + 1) * 128],
                             vals_bf[:, t, :], start=(t == 0), stop=(t == KT - 1))
        out_sb = tailp.tile([128, D], f32, tag="out_sb")
        nc.vector.tensor_scalar(out_sb[:], outp[:], recip[:], None, mult)
        nc.sync.dma_start(out=d_out[:], in_=out_sb[:])

    return nc



def _wait_limit(inst):
    op = inst.get("opcode")
    if op == "Matmult":
        return 1 if inst.get("is_transpose") else 2
    return 1


def _split_excess_waits(raw):
    """Walrus enforces tiny per-instruction sync-wait budgets (1 for most ops,
    2 for Drain/regular Matmult). Tile sometimes emits more (notably the
    kernel-tail drain, which waits on every engine + DMA lane). Hoist the
    excess into preceding same-engine Drain instructions."""
    import json as _json
    d = _json.loads(raw)
    n_split = 0
    for fn in d.get("functions", []):
        for bb in fn.get("blocks", []):
            insts = bb.get("instructions", [])
            out = []
            for inst in insts:
                si = inst.get("sync_info") or {}
                waits = si.get("on_wait") or []
                lim = _wait_limit(inst)
                if len(waits) > lim:
                    excess, keep = waits[:-lim], waits[-lim:]
                    for i, wcmd in enumerate(excess):
                        n_split += 1
                        out.append({
                            "debug": inst.get("debug"),
                            "engine": inst["engine"],
                            "ins": [], "outs": [],
                            "name": f"{inst['name']}-ws{i}",
                            "opcode": "Drain",
                            "sync_info": {"on_wait": [wcmd]},
                        })
                    si["on_wait"] = keep
                    inst["sync_info"] = si
                out.append(inst)
            bb["instructions"] = out
    return _json.dumps(d).encode()


def _patch_json(nc):
    orig = nc.to_json_bytes

    def patched():
        return _split_excess_waits(orig())

    nc.to_json_bytes = patched


def _get_nc():
    if "nc" not in _CACHE:
        nc = _build()
        _patch_json(nc)
        _CACHE["nc"] = nc
    return _CACHE["nc"]


def _run(inputs, trace=False):
    nc = _get_nc()
    query = np.asarray(inputs["query"], dtype=np.float32)
    keys = np.asarray(inputs["keys"], dtype=np.float32)
    values = np.asarray(inputs["values"], dtype=np.float32)
    Wq = np.ascontiguousarray(np.asarray(inputs["Wq"], dtype=np.float32))
    Wk = np.ascontiguousarray(np.asarray(inputs["Wk"], dtype=np.float32))
    v = np.asarray(inputs["v"], dtype=np.float32)

    in_maps = []
    for c in range(NCORE):
        b, qh = c // 2, c % 2
        qw = np.concatenate(
            [query[b, qh * QS:(qh + 1) * QS, :], Wq, Wk, v.reshape(H, 1),
             np.eye(128, dtype=np.float32),
             np.full((128, 1), HALF_PI, dtype=np.float32)], axis=1)
        in_maps.append({
            "qw": np.ascontiguousarray(qw),
            "keys": np.ascontiguousarray(keys[b]),
            "values": np.ascontiguousarray(values[b]),
        })
    res = run_bass_kernel_spmd(nc, in_maps, core_ids=list(range(NCORE)),
                               trace=trace)
    out = np.zeros((B, LQ, D), dtype=np.float32)
    wout = np.zeros((B, LQ, LK), dtype=np.float32)
    for c in range(NCORE):
        b, qh = c // 2, c % 2
        wout[b, qh * QS:(qh + 1) * QS, :] = res.results[c]["wout"]
        out[b, qh * QS:(qh + 1) * QS, :] = res.results[c]["out"]
    return (out, wout), res


def kernel(query, keys, values, Wq, Wk, v):
    (out, wout), _ = _run(dict(query=query, keys=keys, values=values,
                               Wq=Wq, Wk=Wk, v=v))
    return (out, wout)
